# revision 38
# baseline (speedup 1.0000x reference)
"""Trainium2 Bass kernel for the 4-layer spiking (LIF) actor network.

Math (per layer, per timestep; carried states cur/volt/spike):
    cur_t  = 0.5*cur_{t-1} + z_t          z_t = pre_t @ W (+b, b==0 here)
    volt_t = 0.75*volt_{t-1}*(1-s_{t-1}) + cur_t
    s_t    = volt_t > 0.5

Key re-parametrization ("exponential PSUM window"): within a window of
W=17 steps (j = t - t0), a persistent PSUM bank accumulates
    c'_j = 0.5*cur_{t0-1} + sum_{i<=j} 2^i * z_i        (PE accumulation)
so that 2^j*cur_j = c'_j exactly -- the 0.5 decay costs nothing. The
2^j ride on the moving operands: host pre-scales x_t by 2^j; spike ops
emit s" = 0.5*2^j*s (powers of two -> exact in fp16). Scaled states
V = 2^j*volt, R = 2^j*vr (vr := volt*(volt<=0.5)) obey
    V_j = 1.5*R_{j-1} + c'_j              (scalar_tensor_tensor)
    s"  = (V > h_j) * h_j,  h_j = 0.5*2^j (tensor_scalar, 2x DVE mode)
    R_j = (V <= h_j) * V                  (scalar_tensor_tensor)
with V_0 = (0.75*2^-(W-1))*R_old + c'_0 at window boundaries. At each
boundary an ACT copy rescales the bank to cur_base (SBUF) and an fp32
identity-matmul injects 0.5*cur_base as the new window's first matmul.

z matmuls in fp16 hi/lo pairs (1 cyc/row): spikes are exact in fp16
(weights w2..w4 pre-scaled 2x so s"=0.5*2^j*s needs no correction);
weights split w = wh + wl keep ~22 mantissa bits; x split on host into
xh+xl; L1 uses 3 passes xh@wh + xh@wl + xl@wh (dropped xl@wl ~ 2^-22).

Engine balance per step (ns, cost-model): PE ~6200 (28 fp16 z-matmuls
+ amortized window-renorm Id-matmuls), DVE ~5400 (all 6 spikes at 327,
3 volt, 2 vr, L4 tiny ops), Pool ~5640 (3 volt, 4 vr), ACT renorm
copies only. L4 runs in a transposed [128, 8] layout (batch%128 on
partitions, (batch//128, action) on free); z4 is 16 tiny N=2 matmuls
in one windowed accumulation group; sum_t(s4") accumulates via an fp16
128x128 Id matmul.

Sharding: data-parallel over batch across 8 cores; weights replicated.
"""
import sys

sys.path.insert(0, "/opt/trn_rl_repo")
import numpy as np

T, S, H, A = 50, 256, 256, 2
BC = 512  # batch rows per core
NCORES = 8
P = 128
KT = S // P
MT = H // P
W = 17  # renorm window (j in [0, W)); fp16 limit: 0.5*2^16 = 32768
WSTARTS = tuple(range(0, T, W))

_cache: dict = {}


def _win_j(t):
    return t - max(t0 for t0 in WSTARTS if t0 <= t)


def _build(nT=T, dump=0):
    from contextlib import ExitStack

    import concourse.tile as tile
    from concourse import bacc, mybir

    f32 = mybir.dt.float32
    fp16 = mybir.dt.float16
    Alu = mybir.AluOpType
    Act = mybir.ActivationFunctionType

    nc = bacc.Bacc("TRN2", target_bir_lowering=False, debug=False, num_devices=NCORES)
    xhd = nc.dram_tensor("xTh", [T, S, BC], fp16, kind="ExternalInput").ap()
    xld = nc.dram_tensor("xTl", [T, S, BC], fp16, kind="ExternalInput").ap()
    w1hd = nc.dram_tensor("w1h", [S, H], fp16, kind="ExternalInput").ap()
    w1ld = nc.dram_tensor("w1l", [S, H], fp16, kind="ExternalInput").ap()
    w1h2d = nc.dram_tensor("w1h2", [S, H], fp16, kind="ExternalInput").ap()
    w1l2d = nc.dram_tensor("w1l2", [S, H], fp16, kind="ExternalInput").ap()
    wsd = {}
    for l, (rows, cols) in ((2, (H, H)), (3, (H, H)), (4, (H, A))):
        for nm in ("h", "l"):
            wsd[(l, nm)] = nc.dram_tensor(f"w{l}{nm}", [rows, cols], fp16,
                                          kind="ExternalInput").ap()
    id05d = nc.dram_tensor("id05", [P, P], f32, kind="ExternalInput").ap()
    id21d = nc.dram_tensor("id21", [P, P], fp16, kind="ExternalInput").ap()
    outd = nc.dram_tensor("out", [P, 8], f32, kind="ExternalOutput").ap()

    with tile.TileContext(nc) as tc, ExitStack() as ctx:
        consts = ctx.enter_context(tc.tile_pool(name="consts", bufs=1))
        xpool = ctx.enter_context(tc.tile_pool(name="xp", bufs=8))
        vpool = ctx.enter_context(tc.tile_pool(name="volt", bufs=2))
        rpool = ctx.enter_context(tc.tile_pool(name="vr", bufs=2))
        spool = ctx.enter_context(tc.tile_pool(name="sp", bufs=2))
        ipool = ctx.enter_context(tc.tile_pool(name="cimg", bufs=2))
        bpool = ctx.enter_context(tc.tile_pool(name="cbase", bufs=1))
        ppool = ctx.enter_context(tc.tile_pool(name="psum", bufs=1, space="PSUM"))

        # ---- constants: one merged [128, KT*H] tile per weight matrix,
        # loaded with KT row-block DMAs; matmuls slice [128,128] blocks ----
        def _wload(tag, dram, queue):
            t_ = consts.tile([P, KT * H], fp16, tag=tag)
            for k in range(KT):
                queue.dma_start(t_[:, k * H:(k + 1) * H],
                                dram[k * P:(k + 1) * P, :])
            return t_

        w1t = {}  # (hi/lo, k, m) -> [128,128] fp16 lhsT block view
        for nm, dram in (("h", w1hd), ("l", w1ld)):
            mt_ = _wload(f"w1{nm}", dram, nc.gpsimd)
            for k in range(KT):
                for m in range(MT):
                    w1t[(nm, k, m)] = mt_[:, k * H + m * P:k * H + (m + 1) * P]
        # 2x-prescaled w1 blocks for j == 16 (x scale is capped at 2^15:
        # fp16(x) can round up to 1.0, and 1.0 * 2^16 overflows fp16)
        w1t2 = {}
        for nm, dram in (("h2", w1h2d), ("l2", w1l2d)):
            mt_ = _wload(f"w1{nm}", dram, nc.gpsimd)
            for k in range(KT):
                for m in range(MT):
                    w1t2[(nm[0], k, m)] = mt_[:, k * H + m * P:k * H + (m + 1) * P]
        wt = {}  # (layer, term, k, m) -> [128, 128] fp16 lhsT block view
        for l in (2, 3):
            for nm in ("h", "l"):
                mt_ = _wload(f"w{l}{nm}", wsd[(l, nm)], nc.sync)
                for k in range(KT):
                    for m in range(MT):
                        wt[(l, nm, k, m)] = mt_[:, k * H + m * P:k * H + (m + 1) * P]
        w4t = {}
        for k in range(KT):
            for nm in ("h", "l"):
                t_ = consts.tile([P, A], fp16, tag=f"w4{nm}{k}")
                nc.gpsimd.dma_start(t_[:], wsd[(4, nm)][k * P:(k + 1) * P, :])
                w4t[(nm, k)] = t_
        id05 = consts.tile([P, P], f32, tag="id05")
        nc.gpsimd.dma_start(id05[:], id05d[:])
        id128 = consts.tile([P, P], fp16, tag="id21")
        nc.gpsimd.dma_start(id128[:], id21d[:])

        accp = ctx.enter_context(tc.tile_pool(name="accp", bufs=1, space="PSUM"))
        acc = accp.tile([P, 8], f32, tag="acc")

        # persistent window PSUM banks, one per tile
        pwin = {}
        for li in range(3):
            for m in range(MT):
                pw_ = ppool.tile([P, BC], f32, tag=f"P{li}{m}")
                pwin[(li, m)] = pw_
        pw4_ = ppool.tile([P, 8], f32, tag="P4")
        pwin[(3, 0)] = pw4_

        # rotating state refs
        vr = {}    # key -> R = 2^j * vr sbuf tile (fp32)
        sp = {}    # li -> [m0, m1] fp16 spike tiles (s" = 0.5*2^j*s)
        volt = {}
        cbase = {}  # key -> cur_base fp32 tile (written at window ends)

        # zero-init merged R tiles (consumed at t=0)
        for li in range(3):
            zt = rpool.tile([P, 2 * BC], f32, tag=f"vr{li}")
            nc.vector.memset(zt[:], 0.0)
            vr[li] = zt
        z4 = rpool.tile([P, 8], f32, tag="vr3")
        nc.vector.memset(z4[:], 0.0)
        vr[3] = z4
        # L4 keeps an explicit scaled-cur state (its PSUM group is per-step;
        # windowed cross-step accumulation of the sliced-column z4 group
        # mis-accumulates on hardware)
        cpool = ctx.enter_context(tc.tile_pool(name="cur4", bufs=2))
        c4z = cpool.tile([P, 8], f32, tag="cur4")
        nc.vector.memset(c4z[:], 0.0)
        cur4 = [c4z]

        def state_update(li, t, ci=None):
            """merged m0|m1 [128, 2*BC] state ops for li<3 (identical
            scalars for both m-tiles); L4 stays tiny [128, 8].
            V = 1.5*R_old + cur-img; s" = (V > h)*h; R = (V <= h)*V."""
            j = _win_j(t)
            h = 0.5 * 2.0 ** j
            nparts, nfree = (P, 2 * BC) if li < 3 else (P, 8)
            rscal = 1.5 if j > 0 else (0.75 * 2.0 ** -(W - 1))
            if t == 0:
                rscal = 0.0  # R_old is the zero memset tile
            key = li
            vnew = vpool.tile([nparts, nfree], f32, tag=f"volt{li}")
            if li < 3:
                ci = ipool.tile([nparts, nfree], f32, tag=f"ci{li}")
                for m in range(MT):
                    nc.scalar.copy(ci[:, m * BC:(m + 1) * BC],
                                   pwin[(li, m)][:])
                src_ = ci
            else:
                src_ = cur4[0]
            nc.vector.scalar_tensor_tensor(
                vnew[:], vr[key][:], rscal, src_[:], Alu.mult, Alu.add)
            snew = spool.tile([nparts, nfree], fp16, tag=f"sp{li}")
            sig = h if li < 3 else 0.5
            nc.gpsimd.tensor_scalar(snew[:], vnew[:], h, sig,
                                    Alu.is_gt, Alu.mult)
            rnew = rpool.tile([nparts, nfree], f32, tag=f"vr{li}")

            def emit_vr():
                nc.vector.scalar_tensor_tensor(
                    rnew[:], vnew[:], h, vnew[:], Alu.is_le, Alu.mult)

            vr[key] = rnew
            volt[key] = vnew
            return snew, emit_vr

        def renorm(key, li, m, t):
            """at j == W-1: cur_base = 2^-(W-1) * c' (ACT copy to SBUF)."""
            nparts, nfree = (P, BC) if li < 3 else (P, 8)
            cb = bpool.tile([nparts, nfree], f32, tag=f"cb{li}{m}")
            nc.scalar.activation(cb[:], pwin[key][:], Act.Copy,
                                 scale=2.0 ** -(W - 1))
            cbase[key] = cb

        def cell(t, li):
            j = _win_j(t)
            vr_emits = []
            if li < 3:
                l = li + 1
                if l == 1:
                    xt = xpool.tile([P, KT * BC], fp16, tag="x")
                    xt2 = xpool.tile([P, KT * BC], fp16, tag="x")
                    for k in range(KT):
                        nc.sync.dma_start(xt[:, k * BC:(k + 1) * BC],
                                          xhd[t, k * P:(k + 1) * P, :])
                        nc.sync.dma_start(xt2[:, k * BC:(k + 1) * BC],
                                          xld[t, k * P:(k + 1) * P, :])
                    rh_h = [xt[:, k * BC:(k + 1) * BC] for k in range(KT)]
                    rh_l = [xt2[:, k * BC:(k + 1) * BC] for k in range(KT)]
                else:
                    rh_h = sp[li - 1]
                new_sp = []
                for m in range(MT):
                    pt = pwin[(li, m)]
                    mms = []
                    if j == 0 and t > 0:
                        mms.append((id05, cbase[(li, m)]))
                    for k in range(KT):
                        if l == 1:
                            wset = w1t2 if j == W - 1 else w1t
                            mms.append((wset[("h", k, m)], rh_h[k]))
                            mms.append((wset[("h", k, m)], rh_l[k]))
                            mms.append((wset[("l", k, m)], rh_h[k]))
                        else:
                            mms.append((wt[(l, "h", k, m)], rh_h[k]))
                            mms.append((wt[(l, "l", k, m)], rh_h[k]))
                    for i, (lh, rh) in enumerate(mms):
                        nc.tensor.matmul(pt[:], lh[:], rh[:],
                                         start=(j == 0 and i == 0),
                                         stop=(i == len(mms) - 1),
                                         skip_group_check=True)
                    if j == W - 1 and t < nT - 1:
                        renorm((li, m), li, m, t)
                snew, evr = state_update(li, t)
                vr_emits.append(evr)
                sp[li] = [snew[:, m * BC:(m + 1) * BC] for m in range(MT)]
                return vr_emits
            else:
                # transposed L4: z4_tr[b%128, 2*(b//128)+a]; per-step
                # accumulation group (start=True each step): start clears
                # has_written bank-wide, chunks first-touch their columns,
                # lo passes accumulate. Scaled cur4 = 2^j*cur is explicit:
                #   c4*_j = kap*c4*_{j-1} + P4, kap = 1 mid-window.
                rhs = sp[2]
                pt = pwin[(3, 0)]
                mms = []
                for c in range(4):
                    for k in range(KT):
                        mms.append((rhs[k][:, c * P:(c + 1) * P],
                                    w4t[("h", k)], c))
                        mms.append((rhs[k][:, c * P:(c + 1) * P],
                                    w4t[("l", k)], c))
                for i, (lh, rh, c) in enumerate(mms):
                    nc.tensor.matmul(pt[:, 2 * c:2 * c + 2], lh, rh[:],
                                     start=(i == 0),
                                     stop=(i == len(mms) - 1),
                                     skip_group_check=True)
                kap = 1.0 if j > 0 else (0.5 * 2.0 ** -(W - 1) if t > 0 else 0.0)
                c4new = cpool.tile([P, 8], f32, tag="cur4")
                nc.vector.scalar_tensor_tensor(c4new[:], cur4[0][:], kap,
                                               pt[:], Alu.mult, Alu.add)
                cur4[0] = c4new
                snew, evr = state_update(3, t)
                vr_emits.append(evr)
                nc.tensor.matmul(acc[:], id128[:], snew[:], start=(t == 0),
                                 stop=(t == nT - 1), skip_group_check=True)
                return vr_emits

        # descending li: consumers of sp[li-1] must run before cell(t+1, li-1)
        # replaces the python-side reference within the same diagonal
        for d in range(nT + 4):
            deferred = []
            for li in (3, 2, 1, 0):
                t = d - li
                if 0 <= t < nT:
                    deferred.extend(cell(t, li))
                    if dump and t < dump:
                        if li < 3:
                            for m in range(MT):
                                dt_ = nc.dram_tensor(f"dbg_v_{t}_{li}_{m}", [P, BC],
                                                     f32, kind="ExternalOutput").ap()
                                nc.sync.dma_start(dt_[:], volt[li][:, m * BC:(m + 1) * BC])
                        else:
                            dt_ = nc.dram_tensor(f"dbg_v_{t}_3_0", [P, 8], f32,
                                                 kind="ExternalOutput").ap()
                            nc.sync.dma_start(dt_[:], volt[3][:])
            for evr in deferred:
                evr()

        # out = sum_t(s4)/T^2 = acc * 2 / T^2   (acc holds sum of 0.5*s4)
        ot = consts.tile([P, 8], f32, tag="ot")
        nc.scalar.mul(ot[:], acc[:], 2.0 / (T * T))
        nc.sync.dma_start(outd[:], ot[:])

    nc.compile()
    return nc


def _get_nc():
    if "nc" not in _cache:
        _cache["nc"] = _build()
    return _cache["nc"]


def _split_fp16_2(a):
    hi = np.ascontiguousarray(a.astype(np.float16))
    lo = np.ascontiguousarray((a - hi.astype(np.float32)).astype(np.float16))
    return hi, lo


def make_in_maps(x, w1, w2, w3, w4):
    """Host prep: shard x over batch, transpose to [T,S,Bc], split into fp16
    hi+lo and scale each step by 2^j (window position; exact in fp16);
    prescale w2..w4 by 2 (compensates s"=0.5*s) and split into fp16 hi+lo
    pairs; identity blocks."""
    w = {2: 2.0 * np.float32(w2), 3: 2.0 * np.float32(w3),
         4: 2.0 * np.float32(w4)}
    base = {}
    base["w1h"], base["w1l"] = _split_fp16_2(np.float32(w1))
    # 2x variants used at j == W-1 == 16 (x-scale capped at 2^15)
    base["w1h2"] = (base["w1h"].astype(np.float32) * 2).astype(np.float16)
    base["w1l2"] = (base["w1l"].astype(np.float32) * 2).astype(np.float16)
    for l in (2, 3, 4):
        base[f"w{l}h"], base[f"w{l}l"] = _split_fp16_2(w[l])
    base["id05"] = (0.5 * np.eye(P)).astype(np.float32)
    base["id21"] = np.eye(P).astype(np.float16)
    jscale = np.array([2.0 ** min(_win_j(t), W - 2) for t in range(T)],
                      np.float32).reshape(T, 1, 1)
    in_maps = []
    for c in range(NCORES):
        xs = np.asarray(x[c * BC:(c + 1) * BC], np.float32)  # [BC, S, T]
        xTc = xs.transpose(2, 1, 0)                          # [T, S, BC]
        xh, xlo = _split_fp16_2(xTc)
        xh = (xh.astype(np.float32) * jscale).astype(np.float16)
        xlo = (xlo.astype(np.float32) * jscale).astype(np.float16)
        in_maps.append({"xTh": np.ascontiguousarray(xh),
                        "xTl": np.ascontiguousarray(xlo), **base})
    return in_maps


def kernel(x, w1, b1, w2, b2, w3, b3, w4, b4, batch_size):
    from concourse.bass_utils import run_bass_kernel_spmd

    x = np.asarray(x)
    assert x.shape == (NCORES * BC, S, T), x.shape
    # biases are zero in this problem's setup; the kernel folds them out.
    for b in (b1, b2, b3, b4):
        assert np.all(np.asarray(b) == 0.0), "nonzero bias unsupported"
    nc = _get_nc()
    in_maps = make_in_maps(x, np.asarray(w1), np.asarray(w2), np.asarray(w3),
                           np.asarray(w4))
    res = run_bass_kernel_spmd(nc, in_maps, list(range(NCORES)))
    out = np.empty((NCORES * BC, A), np.float32)
    for c in range(NCORES):
        arr = res.results[c]["out"]  # [128, 8]: [p, 2*chunk+a], b = chunk*128+p
        out[c * BC:(c + 1) * BC] = (
            arr.reshape(P, 4, A).transpose(1, 0, 2).reshape(BC, A))
    return out
